# Initial kernel scaffold
#
"""Trainium2 Bass kernel: pre-LN transformer decoder layer on 8 NeuronCores.

Sharding: core = 4*b + g  (b in {0,1} batch, g in {0..3} group rank).
  - Attention: head-parallel (4 of 16 heads per core) over the full batch-b
    sequence; per-core partial attn@woT accumulated via in-group
    ReduceScatter(add) over tokens.
  - FFN: token-parallel (512 tokens per core) with full weights.
All matmuls run as float32r (FP22 multiply, fp32 accumulate).
"""
import math
import numpy as np

import concourse.bacc as bacc
import concourse.bass as bass
import concourse.tile as tile
from concourse import mybir
from concourse.masks import make_identity

B, S, D, H, DH, DFF = 2, 2048, 1024, 16, 64, 4096
G = 4            # cores per batch
LH = H // G      # local heads
LD = LH * DH     # 256 local head dims
SL = S // G      # 512 tokens per core for FFN
P = 128
F32 = mybir.dt.float32
F32R = mybir.dt.float32r
NEG = -1e9

_CACHE = {}


def r(ap):
    return ap.bitcast(F32R)


def build_nc():
    nc = bacc.Bacc("TRN2", target_bir_lowering=False, debug=False, num_devices=8)
    d = {}
    def inp(name, shape):
        d[name] = nc.dram_tensor(name, list(shape), F32, kind="ExternalInput").ap()
    inp("xfull", (S, D))
    inp("xrows", (SL, D))
    inp("wqT", (D, LD)); inp("wkT", (D, LD)); inp("wvT", (D, LD))
    inp("qb", (P, 2)); inp("vb_bc", (P, LD))
    inp("woT", (LD, D)); inp("wob_bc", (P, D))
    inp("mdiag", (P, 16, P)); inp("cmask_bc", (P, S)); inp("rmask16", (P, 16))
    inp("ln1g_bc", (P, D)); inp("ln1b_bc", (P, D))
    inp("ln2g_bc", (P, D)); inp("ln2b_bc", (P, D))
    inp("w1T", (D, DFF)); inp("b1p", (P, DFF // P))
    inp("w2T", (DFF, D)); inp("b2_bc", (P, D))
    out_rows = nc.dram_tensor("out_rows", [SL, D], F32, kind="ExternalOutput").ap()
    partial_d = nc.dram_tensor("partial_d", [S, D], F32).ap()
    rs_d = nc.dram_tensor("rs_d", [SL, D], F32).ap()

    NB = S // P  # 16 token blocks
    DC = D // P  # 8 d chunks

    with tile.TileContext(nc) as tc:
        with tc.tile_pool(name="consts", bufs=1) as consts:
            ident = consts.tile([P, P], F32)
            make_identity(nc, ident)
            eps_sb = consts.tile([P, 1], F32)
            nc.vector.memset(eps_sb, 1e-5)

            qt_cm = tc.tile_pool(name="qt", bufs=1)
            qt_pool = qt_cm.__enter__()
            QT = qt_pool.tile([P, 2, S], F32R)   # q (scaled, +bias), [dh-in-blk, blk, s]
            KT = qt_pool.tile([P, 2, S], F32R)
            V = qt_pool.tile([P, NB, LD], F32R)  # token-major V

            # ---------------- Phase A: LN1 + transpose, Phase B: QKV ----------
            with tc.tile_pool(name="ab", bufs=3) as ab, \
                 tc.tile_pool(name="abw", bufs=1) as abw, \
                 tc.tile_pool(name="xnt_p", bufs=1) as xnt_p, \
                 tc.tile_pool(name="abps", bufs=2, space="PSUM") as abps, \
                 tc.tile_pool(name="qkps", bufs=2, space="PSUM") as qkps:
                ln1g = abw.tile([P, D], F32); nc.sync.dma_start(out=ln1g, in_=d["ln1g_bc"][:])
                ln1b = abw.tile([P, D], F32); nc.sync.dma_start(out=ln1b, in_=d["ln1b_bc"][:])
                wq_sb = abw.tile([P, DC, LD], F32R)
                nc.sync.dma_start(out=wq_sb, in_=r(d["wqT"].rearrange("(c p) o -> p c o", p=P)))
                wk_sb = abw.tile([P, DC, LD], F32R)
                nc.sync.dma_start(out=wk_sb, in_=r(d["wkT"].rearrange("(c p) o -> p c o", p=P)))
                wv_sb = abw.tile([P, DC, LD], F32R)
                nc.sync.dma_start(out=wv_sb, in_=r(d["wvT"].rearrange("(c p) o -> p c o", p=P)))
                qb_sb = abw.tile([P, 2], F32); nc.sync.dma_start(out=qb_sb, in_=d["qb"][:])
                vb_sb = abw.tile([P, LD], F32); nc.sync.dma_start(out=vb_sb, in_=d["vb_bc"][:])
                XNT = xnt_p.tile([P, DC, S], F32R)

                for i in range(NB):
                    xin = ab.tile([P, D], F32, tag="xin")
                    nc.sync.dma_start(out=xin, in_=d["xfull"][i * P:(i + 1) * P, :])
                    stats = ab.tile([P, 2, 6], F32, tag="st")
                    nc.vector.bn_stats(out=stats[:, 0, :], in_=xin[:, 0:512])
                    nc.vector.bn_stats(out=stats[:, 1, :], in_=xin[:, 512:1024])
                    mv = ab.tile([P, 2], F32, tag="mv")
                    nc.vector.bn_aggr(out=mv, in_=stats)
                    rs_t = ab.tile([P, 1], F32, tag="rs")
                    nc.scalar.activation(out=rs_t, in_=mv[:, 1:2],
                                         func=mybir.ActivationFunctionType.Sqrt,
                                         bias=eps_sb)
                    nc.vector.reciprocal(out=rs_t, in_=rs_t)
                    xn = ab.tile([P, D], F32, tag="xn")
                    nc.vector.tensor_scalar(out=xn, in0=xin, scalar1=mv[:, 0:1],
                                            scalar2=rs_t,
                                            op0=mybir.AluOpType.subtract,
                                            op1=mybir.AluOpType.mult)
                    nc.vector.tensor_tensor(out=xn, in0=xn, in1=ln1g,
                                            op=mybir.AluOpType.mult)
                    nc.vector.tensor_tensor(out=xn, in0=xn, in1=ln1b,
                                            op=mybir.AluOpType.add)
                    for dc in range(DC):
                        pt = abps.tile([P, P], F32, tag="tp")
                        nc.tensor.transpose(pt, xn[:, dc * P:(dc + 1) * P], ident)
                        nc.any.tensor_copy(out=XNT[:, dc, i * P:(i + 1) * P], in_=pt)

                # QKV projections
                for pb in range(2):
                    for sc in range(S // 512):
                        psq = qkps.tile([P, 512], F32, tag="psq")
                        psk = qkps.tile([P, 512], F32, tag="psk")
                        for dc in range(DC):
                            nc.tensor.matmul(psq, r(wq_sb[:, dc, pb * P:(pb + 1) * P]),
                                             r(XNT[:, dc, sc * 512:(sc + 1) * 512]),
                                             start=(dc == 0), stop=(dc == DC - 1))
                        for dc in range(DC):
                            nc.tensor.matmul(psk, r(wk_sb[:, dc, pb * P:(pb + 1) * P]),
                                             r(XNT[:, dc, sc * 512:(sc + 1) * 512]),
                                             start=(dc == 0), stop=(dc == DC - 1))
                        nc.scalar.activation(out=QT[:, pb, sc * 512:(sc + 1) * 512],
                                             in_=psq,
                                             func=mybir.ActivationFunctionType.Identity,
                                             bias=qb_sb[:, pb:pb + 1])
                        nc.scalar.activation(out=KT[:, pb, sc * 512:(sc + 1) * 512],
                                             in_=psk,
                                             func=mybir.ActivationFunctionType.Identity)
                for sb in range(NB):
                    psv = qkps.tile([P, LD], F32, tag="psv")
                    for dc in range(DC):
                        nc.tensor.matmul(psv, r(XNT[:, dc, sb * P:(sb + 1) * P]),
                                         r(wv_sb[:, dc, :]),
                                         start=(dc == 0), stop=(dc == DC - 1))
                    nc.vector.tensor_tensor(out=V[:, sb, :], in0=psv, in1=vb_sb,
                                            op=mybir.AluOpType.add)

            # ---------------- Phase C: attention ------------------------------
            with tc.tile_pool(name="cw", bufs=1) as cw, \
                 tc.tile_pool(name="cp", bufs=2) as cp, \
                 tc.tile_pool(name="csm", bufs=3) as csm, \
                 tc.tile_pool(name="c_ps_s", bufs=2, space="PSUM") as c_ps_s, \
                 tc.tile_pool(name="c_ps_t", bufs=2, space="PSUM") as c_ps_t, \
                 tc.tile_pool(name="c_ps_a", bufs=1, space="PSUM") as c_ps_a, \
                 tc.tile_pool(name="c_ps_o", bufs=1, space="PSUM") as c_ps_o:
                wo_sb = cw.tile([64, 4, D], F32R)
                nc.sync.dma_start(out=wo_sb, in_=r(d["woT"].rearrange("(h p) o -> p h o", p=64)))
                md_sb = cw.tile([P, 16, P], F32)
                nc.sync.dma_start(out=md_sb, in_=d["mdiag"][:])
                cm_sb = cw.tile([P, S], F32)
                nc.sync.dma_start(out=cm_sb, in_=d["cmask_bc"][:])
                rm_sb = cw.tile([P, 16], F32)
                nc.sync.dma_start(out=rm_sb, in_=d["rmask16"][:])

                for qi in range(NB):
                    klen = (qi + 1) * P
                    nkc = (klen + 511) // 512
                    part_ps = c_ps_o.tile([P, D], F32, tag="part")
                    for h in range(LH):
                        pb, po = h // 2, (h % 2) * 64
                        p_sb = cp.tile([P, S], F32, tag="p")
                        for kc in range(nkc):
                            n = min(512, klen - kc * 512)
                            ps_s = c_ps_s.tile([P, 512], F32, tag="ps_s")
                            nc.tensor.matmul(
                                ps_s[:, :n],
                                r(QT[po:po + 64, pb, qi * P:(qi + 1) * P]),
                                r(KT[po:po + 64, pb, kc * 512:kc * 512 + n]),
                                start=True, stop=True)
                            nc.vector.scalar_tensor_tensor(
                                out=p_sb[:, kc * 512:kc * 512 + n],
                                in0=ps_s[:, :n],
                                scalar=rm_sb[:, qi:qi + 1],
                                in1=cm_sb[:, kc * 512:kc * 512 + n],
                                op0=mybir.AluOpType.add, op1=mybir.AluOpType.add)
                        nc.vector.tensor_tensor(out=p_sb[:, qi * P:klen],
                                                in0=p_sb[:, qi * P:klen],
                                                in1=md_sb[:, qi, :],
                                                op=mybir.AluOpType.add)
                        mx = csm.tile([P, 1], F32, tag="mx")
                        nc.vector.reduce_max(out=mx, in_=p_sb[:, :klen],
                                             axis=mybir.AxisListType.X)
                        mneg = csm.tile([P, 1], F32, tag="mn")
                        nc.vector.tensor_scalar_mul(mneg, mx, -1.0)
                        ssum = csm.tile([P, 1], F32, tag="ss")
                        nc.scalar.activation(out=p_sb[:, :klen], in_=p_sb[:, :klen],
                                             func=mybir.ActivationFunctionType.Exp,
                                             bias=mneg, accum_out=ssum)
                        rinv = csm.tile([P, 1], F32, tag="ri")
                        nc.vector.reciprocal(out=rinv, in_=ssum)
                        pt_sb = cp.tile([P, S], F32R, tag="pt")
                        for kb in range(qi + 1):
                            tp = c_ps_t.tile([P, P], F32, tag="tp2")
                            nc.tensor.transpose(tp, p_sb[:, kb * P:(kb + 1) * P], ident)
                            nc.any.tensor_copy(out=pt_sb[:, kb * P:(kb + 1) * P], in_=tp)
                        at_ps = c_ps_a.tile([P, 64], F32, tag="at")
                        for kb in range(qi + 1):
                            nc.tensor.matmul(at_ps,
                                             r(pt_sb[:, kb * P:(kb + 1) * P]),
                                             r(V[:, kb, h * DH:(h + 1) * DH]),
                                             start=(kb == 0), stop=(kb == qi))
                        a_sb = csm.tile([P, 64], F32, tag="a")
                        nc.vector.tensor_scalar(out=a_sb, in0=at_ps, scalar1=rinv,
                                                scalar2=None,
                                                op0=mybir.AluOpType.mult)
                        aT_ps = c_ps_a.tile([P, P], F32, tag="aT")
                        nc.tensor.transpose(aT_ps[0:64, :], a_sb, ident)
                        aT_sb = csm.tile([P, P], F32R, tag="aTs")
                        nc.any.tensor_copy(out=aT_sb[0:64, :], in_=aT_ps[0:64, :])
                        for oc in range(2):
                            nc.tensor.matmul(part_ps[:, oc * 512:(oc + 1) * 512],
                                             r(aT_sb[0:64, :]),
                                             r(wo_sb[:, h, oc * 512:(oc + 1) * 512]),
                                             start=(h == 0), stop=(h == LH - 1))
                    part_sb = cp.tile([P, D], F32, tag="part_sb")
                    nc.any.tensor_copy(out=part_sb, in_=part_ps)
                    nc.sync.dma_start(out=partial_d[qi * P:(qi + 1) * P, :], in_=part_sb)

            qt_cm.__exit__(None, None, None)

            # ---------------- ReduceScatter -----------------------------------
            nc.gpsimd.collective_compute(
                "ReduceScatter", mybir.AluOpType.add,
                replica_groups=[[0, 1, 2, 3], [4, 5, 6, 7]],
                ins=[partial_d[:]], outs=[rs_d[:]])

            # ---------------- Phase D: residual + LN2 + FFN -------------------
            with tc.tile_pool(name="dw", bufs=1) as dw, \
                 tc.tile_pool(name="dt", bufs=3) as dt, \
                 tc.tile_pool(name="dxp", bufs=1) as dxp, \
                 tc.tile_pool(name="dw1", bufs=3) as dw1, \
                 tc.tile_pool(name="dh", bufs=1) as dh_p, \
                 tc.tile_pool(name="dw2", bufs=3) as dw2_p, \
                 tc.tile_pool(name="d_ps_h", bufs=2, space="PSUM") as d_ps_h, \
                 tc.tile_pool(name="d_ps_t", bufs=2, space="PSUM") as d_ps_t, \
                 tc.tile_pool(name="d_ps_o", bufs=1, space="PSUM") as d_ps_o:
                wob = dw.tile([P, D], F32); nc.sync.dma_start(out=wob, in_=d["wob_bc"][:])
                ln2g = dw.tile([P, D], F32); nc.sync.dma_start(out=ln2g, in_=d["ln2g_bc"][:])
                ln2b = dw.tile([P, D], F32); nc.sync.dma_start(out=ln2b, in_=d["ln2b_bc"][:])
                b1_sb = dw.tile([P, DFF // P], F32); nc.sync.dma_start(out=b1_sb, in_=d["b1p"][:])
                b2_sb = dw.tile([P, D], F32); nc.sync.dma_start(out=b2_sb, in_=d["b2_bc"][:])
                XP = dxp.tile([P, 4, D], F32)   # X' rows (post-attn residual)
                YNT = dxp.tile([P, DC, SL], F32R)
                HT = dh_p.tile([P, DFF // P, SL], F32R)

                for sb in range(4):
                    rs_sb = dt.tile([P, D], F32, tag="rs_in")
                    nc.sync.dma_start(out=rs_sb, in_=rs_d[sb * P:(sb + 1) * P, :])
                    xr_sb = dt.tile([P, D], F32, tag="xr")
                    nc.sync.dma_start(out=xr_sb, in_=d["xrows"][sb * P:(sb + 1) * P, :])
                    nc.vector.tensor_tensor(out=rs_sb, in0=rs_sb, in1=xr_sb,
                                            op=mybir.AluOpType.add)
                    nc.vector.tensor_tensor(out=XP[:, sb, :], in0=rs_sb, in1=wob,
                                            op=mybir.AluOpType.add)
                    stats = dt.tile([P, 2, 6], F32, tag="st2")
                    nc.vector.bn_stats(out=stats[:, 0, :], in_=XP[:, sb, 0:512])
                    nc.vector.bn_stats(out=stats[:, 1, :], in_=XP[:, sb, 512:1024])
                    mv = dt.tile([P, 2], F32, tag="mv2")
                    nc.vector.bn_aggr(out=mv, in_=stats)
                    rs_t = dt.tile([P, 1], F32, tag="rs2")
                    nc.scalar.activation(out=rs_t, in_=mv[:, 1:2],
                                         func=mybir.ActivationFunctionType.Sqrt,
                                         bias=eps_sb)
                    nc.vector.reciprocal(out=rs_t, in_=rs_t)
                    yn = dt.tile([P, D], F32, tag="yn")
                    nc.vector.tensor_scalar(out=yn, in0=XP[:, sb, :], scalar1=mv[:, 0:1],
                                            scalar2=rs_t,
                                            op0=mybir.AluOpType.subtract,
                                            op1=mybir.AluOpType.mult)
                    nc.vector.tensor_tensor(out=yn, in0=yn, in1=ln2g,
                                            op=mybir.AluOpType.mult)
                    nc.vector.tensor_tensor(out=yn, in0=yn, in1=ln2b,
                                            op=mybir.AluOpType.add)
                    for dc in range(DC):
                        tp = d_ps_t.tile([P, P], F32, tag="tp3")
                        nc.tensor.transpose(tp, yn[:, dc * P:(dc + 1) * P], ident)
                        nc.any.tensor_copy(out=YNT[:, dc, sb * P:(sb + 1) * P], in_=tp)

                w1r = d["w1T"].rearrange("(c p) m -> p c m", p=P)
                w2r = d["w2T"].rearrange("(c p) o -> p c o", p=P)
                for c in range(DFF // P):
                    w1_sb = dw1.tile([P, DC, P], F32R, tag="w1")
                    nc.sync.dma_start(out=w1_sb, in_=r(w1r[:, :, c * P:(c + 1) * P]))
                    ps_h = d_ps_h.tile([P, SL], F32, tag="ps_h")
                    for dc in range(DC):
                        nc.tensor.matmul(ps_h, r(w1_sb[:, dc, :]),
                                         r(YNT[:, dc, :]),
                                         start=(dc == 0), stop=(dc == DC - 1))
                    nc.scalar.activation(out=HT[:, c, :], in_=ps_h,
                                         func=mybir.ActivationFunctionType.Gelu,
                                         bias=b1_sb[:, c:c + 1])

                for oc in range(2):
                    ps_os = [d_ps_o.tile([P, 512], F32, tag=f"ps_o{sb}", name=f"ps_o{sb}")
                             for sb in range(4)]
                    for c in range(DFF // P):
                        w2c = dw2_p.tile([P, 512], F32R, tag="w2c")
                        nc.sync.dma_start(out=w2c,
                                          in_=r(w2r[:, c, oc * 512:(oc + 1) * 512]))
                        for sb in range(4):
                            nc.tensor.matmul(
                                ps_os[sb], r(HT[:, c, sb * P:(sb + 1) * P]),
                                r(w2c),
                                start=(c == 0), stop=(c == DFF // P - 1))
                    for sb in range(4):
                        fin = dt.tile([P, 512], F32, tag="fin")
                        nc.vector.tensor_tensor(out=fin, in0=ps_os[sb],
                                                in1=b2_sb[:, oc * 512:(oc + 1) * 512],
                                                op=mybir.AluOpType.add)
                        nc.vector.tensor_tensor(out=fin, in0=fin,
                                                in1=XP[:, sb, oc * 512:(oc + 1) * 512],
                                                op=mybir.AluOpType.add)
                        nc.sync.dma_start(
                            out=out_rows[sb * P:(sb + 1) * P, oc * 512:(oc + 1) * 512],
                            in_=fin)

    nc.compile()
    return nc


def make_in_maps(X, mask, valid_lens, wq_w, wq_b, wk_w, wv_w, wv_b, wo_w, wo_b,
                 ln1_g, ln1_b, ln2_g, ln2_b, w1, b1, w2, b2):
    f = np.float32
    bc = lambda v: np.broadcast_to(np.asarray(v, f)[None, :], (P, len(v))).copy()
    mdiag = np.stack([mask[i * P:(i + 1) * P, i * P:(i + 1) * P] for i in range(16)])
    mdiag = np.ascontiguousarray(mdiag.transpose(1, 0, 2)).astype(f)
    idx = np.arange(S)
    in_maps = []
    for core in range(8):
        b, g = core // G, core % G
        vmask1 = np.where(idx >= valid_lens[b], NEG, 0.0).astype(f)
        hs = slice(g * LD, (g + 1) * LD)
        m = {
            "xfull": np.ascontiguousarray(X[b]).astype(f),
            "xrows": np.ascontiguousarray(X[b, g * SL:(g + 1) * SL]).astype(f),
            "wqT": np.ascontiguousarray((wq_w[hs, :] * 0.125).T).astype(f),
            "wkT": np.ascontiguousarray(wk_w[hs, :].T).astype(f),
            "wvT": np.ascontiguousarray(wv_w[hs, :].T).astype(f),
            "qb": np.ascontiguousarray((wq_b[hs] * 0.125).reshape(2, P).T).astype(f),
            "vb_bc": bc(wv_b[hs]),
            "woT": np.ascontiguousarray(wo_w.T[hs, :]).astype(f),
            "wob_bc": bc(wo_b),
            "mdiag": mdiag,
            "cmask_bc": bc(vmask1),
            "rmask16": np.ascontiguousarray(vmask1.reshape(16, P).T).astype(f),
            "ln1g_bc": bc(ln1_g), "ln1b_bc": bc(ln1_b),
            "ln2g_bc": bc(ln2_g), "ln2b_bc": bc(ln2_b),
            "w1T": np.ascontiguousarray(w1.T).astype(f),
            "b1p": np.ascontiguousarray(b1.reshape(DFF // P, P).T).astype(f),
            "w2T": np.ascontiguousarray(w2.T).astype(f),
            "b2_bc": bc(b2),
        }
        in_maps.append(m)
    return in_maps


def kernel(**inputs):
    from concourse.bass_utils import run_bass_kernel_spmd
    if "nc" not in _CACHE:
        _CACHE["nc"] = build_nc()
    nc = _CACHE["nc"]
    in_maps = make_in_maps(**inputs)
    res = run_bass_kernel_spmd(nc, in_maps, list(range(8)))
    out = np.empty((B, S, D), np.float32)
    for core in range(8):
        b, g = core // G, core % G
        out[b, g * SL:(g + 1) * SL, :] = res.results[core]["out_rows"]
    return out



# revision 44
# speedup vs baseline: 1.7276x; 1.7276x over previous
"""Trainium2 Bass kernel: pre-LN transformer decoder layer on 8 NeuronCores.

Sharding: core = 4*b + g  (b in {0,1} batch, g in {0..3} group rank).
  - Attention: head-parallel (4 of 16 heads per core) over the full batch-b
    sequence, computed in a TRANSPOSED-score formulation:
      scores_T[k, q] = K_blk @ Q^T   (softmax needs no max-subtraction here;
      scores are O(6)), exp on ScalarE with the key-padding mask as a
      per-partition bias, denominator via an appended ones-column on V,
      so no P-transposes and no reductions at all.  attn output comes out
      already transposed [dh, q] — perfect for the output projection.
  - Output projection partials (4 local heads) are computed per 512-token
    group and exchanged with chunked bf16 ReduceScatters (1 MB each),
    overlapped with the attention compute of later token groups.
  - FFN: token-parallel (512 tokens per core, strided 128-blocks) with
    full weights (bf16).
All matmul operands are bf16 (fp32 accumulation in PSUM).
"""
import math
import numpy as np
import ml_dtypes

import concourse.bacc as bacc
import concourse.bass as bass
import concourse.tile as tile
from concourse import mybir
from concourse.masks import make_identity

B, S, D, H, DH, DFF = 2, 2048, 1024, 16, 64, 4096
G = 4            # cores per batch
LH = H // G      # local heads
LD = LH * DH     # 256 local head dims
SL = S // G      # 512 tokens per core for FFN / output projection
P = 128
NB = S // P      # 16 token blocks
DC = D // P      # 8 d chunks
F32 = mybir.dt.float32
F32R = mybir.dt.float32r
BF16 = mybir.dt.bfloat16
NEG = -1e9
NPBF16 = ml_dtypes.bfloat16

_CACHE = {}


def r(ap):
    return ap.bitcast(F32R)


def build_nc():
    nc = bacc.Bacc("TRN2", target_bir_lowering=False, debug=False, num_devices=8)
    d = {}
    def inp(name, shape, dt=F32):
        d[name] = nc.dram_tensor(name, list(shape), dt, kind="ExternalInput").ap()
    inp("xfull", (S, D))
    inp("xrows", (SL, D))
    inp("wqT", (D, LD), BF16); inp("wkT", (D, LD), BF16); inp("wvT", (D, LD), BF16)
    inp("qb", (P, 2)); inp("kb2", (P, 2)); inp("vb_bc", (P, LD))
    inp("wo_pair", (P, 2, D), BF16); inp("wob_bc", (P, D))
    inp("mdiagT", (P, NB, P)); inp("rmask16", (P, NB))
    inp("ln1g_bc", (P, D)); inp("ln2g_bc", (P, D))
    inp("w1T", (D, DFF), BF16); inp("b1p", (P, DFF // P))
    inp("w2T", (DFF, D), BF16); inp("b2_bc", (P, D))
    out_rows = nc.dram_tensor("out_rows", [SL, D], F32, kind="ExternalOutput").ap()
    partial_d = nc.dram_tensor("partial_d", [S, D], BF16).ap()
    rs_d = nc.dram_tensor("rs_d", [SL, D], BF16).ap()

    with tile.TileContext(nc) as tc:
        with tc.tile_pool(name="consts", bufs=1) as consts:
            identb = consts.tile([P, P], BF16)
            make_identity(nc, identb)
            eps_sb = consts.tile([P, 1], F32)
            nc.vector.memset(eps_sb, 1e-5)
            ones1f = consts.tile([1, 64], F32)
            nc.vector.memset(ones1f, 1.0)
            ones1 = consts.tile([1, 64], F32R)
            nc.any.tensor_copy(out=ones1, in_=ones1f)

            w1_cm = tc.tile_pool(name="w1p", bufs=1)
            w1_pool = w1_cm.__enter__()
            w1full = w1_pool.tile([P, DC, DFF], BF16)
            qt_cm = tc.tile_pool(name="qt", bufs=1)
            qt_pool = qt_cm.__enter__()
            QT = qt_pool.tile([P, 2, S], BF16)      # [dh-in-pair, pb, s] (q scaled+bias)
            KT = qt_pool.tile([P, 2, S], BF16)
            V = qt_pool.tile([P, NB, LH, DH + 1], BF16)   # token-major V + ones col
            nc.vector.memset(V[:, :, :, DH:DH + 1], 1.0)

            # ---------------- Phase A: LN1 + transpose, Phase B: QKV ----------
            with tc.tile_pool(name="ab", bufs=3) as ab, \
                 tc.tile_pool(name="abw", bufs=1) as abw, \
                 tc.tile_pool(name="xnt_p", bufs=1) as xnt_p, \
                 tc.tile_pool(name="abps", bufs=2, space="PSUM") as abps, \
                 tc.tile_pool(name="qkps", bufs=2, space="PSUM") as qkps:
                ln1g = abw.tile([P, D], F32); nc.sync.dma_start(out=ln1g, in_=d["ln1g_bc"][:])
                wq_sb = abw.tile([P, DC, LD], BF16)
                nc.sync.dma_start(out=wq_sb, in_=d["wqT"].rearrange("(c p) o -> p c o", p=P))
                wk_sb = abw.tile([P, DC, LD], BF16)
                nc.sync.dma_start(out=wk_sb, in_=d["wkT"].rearrange("(c p) o -> p c o", p=P))
                wv_sb = abw.tile([P, DC, LD], BF16)
                nc.sync.dma_start(out=wv_sb, in_=d["wvT"].rearrange("(c p) o -> p c o", p=P))
                qb_sb = abw.tile([P, 2], F32); nc.sync.dma_start(out=qb_sb, in_=d["qb"][:])
                kb_sb = abw.tile([P, 2], F32); nc.sync.dma_start(out=kb_sb, in_=d["kb2"][:])
                vb_sb = abw.tile([P, LD], F32); nc.sync.dma_start(out=vb_sb, in_=d["vb_bc"][:])
                w1rr = d["w1T"].rearrange("(c p) m -> p c m", p=P)
                for c in range(DC):
                    nc.sync.dma_start(out=w1full[:, c, :], in_=w1rr[:, c, :])
                XNT = xnt_p.tile([P, DC, S], BF16)

                for i in range(NB):
                    xin = ab.tile([P, D], F32, tag="xin")
                    nc.sync.dma_start(out=xin, in_=d["xfull"][i * P:(i + 1) * P, :])
                    stats = ab.tile([P, 2, 6], F32, tag="st")
                    nc.vector.bn_stats(out=stats[:, 0, :], in_=xin[:, 0:512])
                    nc.vector.bn_stats(out=stats[:, 1, :], in_=xin[:, 512:1024])
                    mv = ab.tile([P, 2], F32, tag="mv")
                    nc.vector.bn_aggr(out=mv, in_=stats)
                    rs_t = ab.tile([P, 1], F32, tag="rs")
                    nc.scalar.activation(out=rs_t, in_=mv[:, 1:2],
                                         func=mybir.ActivationFunctionType.Sqrt,
                                         bias=eps_sb)
                    nc.vector.reciprocal(out=rs_t, in_=rs_t)
                    nmurs = ab.tile([P, 1], F32, tag="nm")
                    nc.vector.tensor_tensor(out=nmurs, in0=mv[:, 0:1], in1=rs_t,
                                            op=mybir.AluOpType.mult)
                    nc.vector.tensor_scalar_mul(nmurs, nmurs, -1.0)
                    xn = ab.tile([P, D], F32, tag="xn")
                    nc.scalar.activation(out=xn, in_=xin,
                                         func=mybir.ActivationFunctionType.Identity,
                                         scale=rs_t, bias=nmurs)
                    xnb = ab.tile([P, D], BF16, tag="xnb")
                    nc.vector.tensor_tensor(out=xnb, in0=xn, in1=ln1g,
                                            op=mybir.AluOpType.mult)
                    for dc in range(DC):
                        pt = abps.tile([P, P], BF16, tag="tp")
                        nc.tensor.transpose(pt, xnb[:, dc * P:(dc + 1) * P], identb)
                        nc.any.tensor_copy(out=XNT[:, dc, i * P:(i + 1) * P], in_=pt)

                # QKV projections
                for pb in range(2):
                    for sc in range(S // 512):
                        psq = qkps.tile([P, 512], F32, tag="psq")
                        psk = qkps.tile([P, 512], F32, tag="psk")
                        for dc in range(DC):
                            nc.tensor.matmul(psq, wq_sb[:, dc, pb * P:(pb + 1) * P],
                                             XNT[:, dc, sc * 512:(sc + 1) * 512],
                                             start=(dc == 0), stop=(dc == DC - 1))
                        for dc in range(DC):
                            nc.tensor.matmul(psk, wk_sb[:, dc, pb * P:(pb + 1) * P],
                                             XNT[:, dc, sc * 512:(sc + 1) * 512],
                                             start=(dc == 0), stop=(dc == DC - 1))
                        nc.scalar.activation(out=QT[:, pb, sc * 512:(sc + 1) * 512],
                                             in_=psq,
                                             func=mybir.ActivationFunctionType.Identity,
                                             bias=qb_sb[:, pb:pb + 1])
                        nc.scalar.activation(out=KT[:, pb, sc * 512:(sc + 1) * 512],
                                             in_=psk,
                                             func=mybir.ActivationFunctionType.Identity,
                                             bias=kb_sb[:, pb:pb + 1])
                for sb in range(NB):
                    psv = qkps.tile([P, LD], F32, tag="psv")
                    for dc in range(DC):
                        nc.tensor.matmul(psv, XNT[:, dc, sb * P:(sb + 1) * P],
                                         wv_sb[:, dc, :],
                                         start=(dc == 0), stop=(dc == DC - 1))
                    nc.vector.tensor_tensor(
                        out=V[:, sb, :, 0:DH],
                        in0=psv[:].rearrange("p (h e) -> p h e", h=LH),
                        in1=vb_sb[:].rearrange("p (h e) -> p h e", h=LH),
                        op=mybir.AluOpType.add)

            # ---------------- Phase C: attention (transposed scores) ----------
            with tc.tile_pool(name="cw", bufs=1) as cw, \
                 tc.tile_pool(name="cat", bufs=2) as cat, \
                 tc.tile_pool(name="cee", bufs=3) as cee, \
                 tc.tile_pool(name="csm", bufs=2) as csm, \
                 tc.tile_pool(name="c_ps_s", bufs=2, space="PSUM") as c_ps_s, \
                 tc.tile_pool(name="c_ps_at", bufs=1, space="PSUM") as c_ps_at, \
                 tc.tile_pool(name="c_ps_rb", bufs=1, space="PSUM") as c_ps_rb:
                wo_sb = cw.tile([P, 2, D], BF16)
                nc.sync.dma_start(out=wo_sb, in_=d["wo_pair"][:])
                mdT = cw.tile([P, NB, P], F32)
                nc.sync.dma_start(out=mdT, in_=d["mdiagT"][:])
                rm_sb = cw.tile([P, NB], F32)
                nc.sync.dma_start(out=rm_sb, in_=d["rmask16"][:])

                for gq in range(4):
                    ATg = cat.tile([P, 2, 512], BF16, tag="ATg")
                    for pb in range(2):   # head pair (2*pb, 2*pb+1), row-packed
                        at0 = c_ps_at.tile([DH + 1, 512], F32, tag="at0")
                        at1 = c_ps_at.tile([DH + 1, 512], F32, tag="at1")
                        ats = (at0, at1)
                        for kb in range(4 * gq + 4):
                            q0, qe = max(kb, 4 * gq), 4 * gq + 4
                            w = (qe - q0) * P
                            col0 = (q0 - 4 * gq) * P
                            ps0 = c_ps_s.tile([P, 512], F32, tag="ps0")
                            ps1 = c_ps_s.tile([P, 512], F32, tag="ps1")
                            pss = (ps0, ps1)
                            # two heads in different PE row-groups -> concurrent
                            for i, po in enumerate((0, 64)):
                                nc.tensor.matmul(
                                    pss[i][:, :w],
                                    KT[po:po + 64, pb, kb * P:(kb + 1) * P],
                                    QT[po:po + 64, pb, q0 * P:qe * P],
                                    start=True, stop=True)
                            for i in range(2):
                                if q0 == kb:
                                    nc.vector.tensor_tensor(out=pss[i][:, 0:P],
                                                            in0=pss[i][:, 0:P],
                                                            in1=mdT[:, kb, :],
                                                            op=mybir.AluOpType.add)
                                ept = cee.tile([P, 512], BF16, tag="ept")
                                nc.scalar.activation(
                                    out=ept[:, :w], in_=pss[i][:, :w],
                                    func=mybir.ActivationFunctionType.Exp,
                                    bias=rm_sb[:, kb:kb + 1])
                                Vst = V[:, kb, 2 * pb + i, :]
                                if q0 == kb:
                                    nc.tensor.matmul(ats[i][:, col0:col0 + P], Vst,
                                                     ept[:, 0:P],
                                                     start=(kb == 0), stop=True)
                                    if w > P:
                                        nc.tensor.matmul(ats[i][:, col0 + P:col0 + w],
                                                         Vst, ept[:, P:w],
                                                         start=(kb == 0), stop=False)
                                else:
                                    nc.tensor.matmul(ats[i][:, 0:w], Vst, ept[:, :w],
                                                     start=(kb == 0), stop=False)
                        for i in range(2):
                            rinv = csm.tile([1, 512], F32, tag="ri")
                            nc.vector.reciprocal(out=rinv, in_=ats[i][DH:DH + 1, :])
                            rinvb = csm.tile([1, 512], F32R, tag="rib")
                            nc.any.tensor_copy(out=rinvb, in_=rinv)
                            rb_ps = c_ps_rb.tile([64, 512], F32, tag="rb")
                            nc.tensor.matmul(rb_ps, ones1, rinvb,
                                             start=True, stop=True)
                            rb_sb = csm.tile([64, 512], F32, tag="rbs")
                            nc.any.tensor_copy(out=rb_sb, in_=rb_ps)
                            nc.vector.tensor_tensor(out=ATg[i * 64:(i + 1) * 64, pb, :],
                                                    in0=ats[i][0:DH, :], in1=rb_sb,
                                                    op=mybir.AluOpType.mult)
                    # output-projection partials for this token group (bf16)
                    for qi2 in range(4):
                        row0 = (gq * 4 + qi2) * P
                        for oc in range(2):
                            pp = c_ps_rb.tile([P, 512], F32, tag="pp")
                            for hp in range(2):
                                nc.tensor.matmul(
                                    pp, ATg[:, hp, qi2 * P:(qi2 + 1) * P],
                                    wo_sb[:, hp, oc * 512:(oc + 1) * 512],
                                    start=(hp == 0), stop=(hp == 1))
                            ppb = cee.tile([P, 512], BF16, tag="ppb")
                            nc.any.tensor_copy(out=ppb, in_=pp)
                            nc.sync.dma_start(
                                out=partial_d[row0:row0 + P, oc * 512:(oc + 1) * 512],
                                in_=ppb)
                    nc.gpsimd.collective_compute(
                        "ReduceScatter", mybir.AluOpType.add,
                        replica_groups=[[0, 1, 2, 3], [4, 5, 6, 7]],
                        ins=[partial_d[gq * 512:(gq + 1) * 512, :]],
                        outs=[rs_d[gq * P:(gq + 1) * P, :]])

            qt_cm.__exit__(None, None, None)

            # ---------------- Phase D: output proj + residual + LN2 + FFN -----
            with tc.tile_pool(name="dw", bufs=1) as dw, \
                 tc.tile_pool(name="dt", bufs=3) as dt, \
                 tc.tile_pool(name="dxp", bufs=1) as dxp, \
                 tc.tile_pool(name="dh", bufs=1) as dh_p, \
                 tc.tile_pool(name="dw2", bufs=3) as dw2_p, \
                 tc.tile_pool(name="d_ps_h", bufs=2, space="PSUM") as d_ps_h, \
                 tc.tile_pool(name="d_ps_t", bufs=2, space="PSUM") as d_ps_t, \
                 tc.tile_pool(name="d_ps_o", bufs=1, space="PSUM") as d_ps_o:
                xr_sb = dw.tile([P, 4, D], F32)
                nc.sync.dma_start(out=xr_sb, in_=d["xrows"].rearrange("(s p) d -> p s d", p=P))
                wob = dw.tile([P, D], F32); nc.sync.dma_start(out=wob, in_=d["wob_bc"][:])
                ln2g = dw.tile([P, D], F32); nc.sync.dma_start(out=ln2g, in_=d["ln2g_bc"][:])
                b1_sb = dw.tile([P, DFF // P], F32); nc.sync.dma_start(out=b1_sb, in_=d["b1p"][:])
                b2_sb = dw.tile([P, D], F32); nc.sync.dma_start(out=b2_sb, in_=d["b2_bc"][:])
                XP = dxp.tile([P, 4, D], F32)   # X' rows (post-attn residual)
                YNT = dxp.tile([P, DC, SL], BF16)
                HT = dh_p.tile([P, DFF // P, SL], BF16)

                for sb in range(4):
                    rs_sb = dt.tile([P, D], BF16, tag="rs_in")
                    nc.sync.dma_start(out=rs_sb, in_=rs_d[sb * P:(sb + 1) * P, :])
                    tmp = dt.tile([P, D], F32, tag="wtmp")
                    nc.vector.tensor_tensor(out=tmp, in0=rs_sb, in1=xr_sb[:, sb, :],
                                            op=mybir.AluOpType.add)
                    nc.vector.tensor_tensor(out=XP[:, sb, :], in0=tmp, in1=wob,
                                            op=mybir.AluOpType.add)
                    stats = dt.tile([P, 2, 6], F32, tag="st2")
                    nc.vector.bn_stats(out=stats[:, 0, :], in_=XP[:, sb, 0:512])
                    nc.vector.bn_stats(out=stats[:, 1, :], in_=XP[:, sb, 512:1024])
                    mv = dt.tile([P, 2], F32, tag="mv2")
                    nc.vector.bn_aggr(out=mv, in_=stats)
                    rs_t = dt.tile([P, 1], F32, tag="rs2")
                    nc.scalar.activation(out=rs_t, in_=mv[:, 1:2],
                                         func=mybir.ActivationFunctionType.Sqrt,
                                         bias=eps_sb)
                    nc.vector.reciprocal(out=rs_t, in_=rs_t)
                    nmurs = dt.tile([P, 1], F32, tag="nm2")
                    nc.vector.tensor_tensor(out=nmurs, in0=mv[:, 0:1], in1=rs_t,
                                            op=mybir.AluOpType.mult)
                    nc.vector.tensor_scalar_mul(nmurs, nmurs, -1.0)
                    yn = dt.tile([P, D], F32, tag="yn")
                    nc.scalar.activation(out=yn, in_=XP[:, sb, :],
                                         func=mybir.ActivationFunctionType.Identity,
                                         scale=rs_t, bias=nmurs)
                    ynb = dt.tile([P, D], BF16, tag="ynb")
                    nc.vector.tensor_tensor(out=ynb, in0=yn, in1=ln2g,
                                            op=mybir.AluOpType.mult)
                    for dc in range(DC):
                        tp = d_ps_t.tile([P, P], BF16, tag="tp3")
                        nc.tensor.transpose(tp, ynb[:, dc * P:(dc + 1) * P], identb)
                        nc.any.tensor_copy(out=YNT[:, dc, sb * P:(sb + 1) * P], in_=tp)

                w2r = d["w2T"].rearrange("(c p) o -> p c o", p=P)
                for c in range(DFF // P):
                    ps_h = d_ps_h.tile([P, SL], F32, tag="ps_h")
                    for dc in range(DC):
                        nc.tensor.matmul(ps_h, w1full[:, dc, c * P:(c + 1) * P],
                                         YNT[:, dc, :],
                                         start=(dc == 0), stop=(dc == DC - 1))
                    nc.scalar.activation(out=HT[:, c, :], in_=ps_h,
                                         func=mybir.ActivationFunctionType.Gelu,
                                         bias=b1_sb[:, c:c + 1])

                for oc in range(2):
                    ps_os = [d_ps_o.tile([P, 512], F32, tag=f"ps_o{sb}", name=f"ps_o{sb}")
                             for sb in range(4)]
                    for c in range(DFF // P):
                        w2c = dw2_p.tile([P, 512], BF16, tag="w2c")
                        nc.sync.dma_start(out=w2c,
                                          in_=w2r[:, c, oc * 512:(oc + 1) * 512])
                        for sb in range(4):
                            nc.tensor.matmul(
                                ps_os[sb], HT[:, c, sb * P:(sb + 1) * P],
                                w2c,
                                start=(c == 0), stop=(c == DFF // P - 1))
                    for sb in range(4):
                        fin = dt.tile([P, 512], F32, tag="fin")
                        nc.vector.tensor_tensor(out=fin, in0=ps_os[sb],
                                                in1=b2_sb[:, oc * 512:(oc + 1) * 512],
                                                op=mybir.AluOpType.add)
                        nc.vector.tensor_tensor(out=fin, in0=fin,
                                                in1=XP[:, sb, oc * 512:(oc + 1) * 512],
                                                op=mybir.AluOpType.add)
                        nc.sync.dma_start(
                            out=out_rows[sb * P:(sb + 1) * P, oc * 512:(oc + 1) * 512],
                            in_=fin)

            w1_cm.__exit__(None, None, None)

    nc.compile()
    return nc


def make_in_maps(X, mask, valid_lens, wq_w, wq_b, wk_w, wv_w, wv_b, wo_w, wo_b,
                 ln1_g, ln1_b, ln2_g, ln2_b, w1, b1, w2, b2):
    f = np.float32
    bc = lambda v: np.broadcast_to(np.asarray(v, f)[None, :], (P, len(v))).copy()
    # transposed causal diagonal blocks: mdT[kk, i, qq] = mask[i*P+qq, i*P+kk]
    mdT = np.stack([np.asarray(mask[i * P:(i + 1) * P, i * P:(i + 1) * P]).T
                    for i in range(NB)])            # [i, kk, qq]
    mdT = np.ascontiguousarray(mdT.transpose(1, 0, 2)).astype(f)   # [kk, i, qq]
    idx = np.arange(S)
    woT = np.ascontiguousarray(np.asarray(wo_w, f).T)   # [d_in, d_out]
    p_ar = np.arange(P)
    in_maps = []
    for core in range(8):
        b, g = core // G, core % G
        vmask1 = np.where(idx >= valid_lens[b], NEG, 0.0).astype(f)
        hs = slice(g * LD, (g + 1) * LD)
        # local-head-pair layout matching ATg: partition p = (h%2)*64 + dh,
        # second dim hp = h//2, local head h = hp*2 + p//64
        wo_pair = np.empty((P, 2, D), NPBF16)
        for hp in range(2):
            head = g * 4 + hp * 2 + p_ar // 64
            rows = head * 64 + (p_ar % 64)
            wo_pair[:, hp, :] = woT[rows, :].astype(NPBF16)
        # strided token ownership: rows sb*512 + g*128 .. +128, sb = 0..3
        own = np.concatenate([np.arange(sb * 512 + g * P, sb * 512 + (g + 1) * P)
                              for sb in range(4)])
        m = {
            "xfull": np.ascontiguousarray(X[b]).astype(f),
            "xrows": np.ascontiguousarray(X[b][own]).astype(f),
            "wqT": np.ascontiguousarray((wq_w[hs, :] * 0.125).T).astype(NPBF16),
            "wkT": np.ascontiguousarray(wk_w[hs, :].T).astype(NPBF16),
            "wvT": np.ascontiguousarray(wv_w[hs, :].T).astype(NPBF16),
            "qb": np.ascontiguousarray(
                (0.125 * (wq_b[hs] + ln1_b @ np.asarray(wq_w, f)[hs].T))
                .reshape(2, P).T).astype(f),
            "kb2": np.ascontiguousarray(
                (ln1_b @ np.asarray(wk_w, f)[hs].T).reshape(2, P).T).astype(f),
            "vb_bc": bc(wv_b[hs] + ln1_b @ np.asarray(wv_w, f)[hs].T),
            "wo_pair": wo_pair,
            "wob_bc": bc(wo_b),
            "mdiagT": mdT,
            "rmask16": np.ascontiguousarray(vmask1.reshape(NB, P).T).astype(f),
            "ln1g_bc": bc(ln1_g), "ln2g_bc": bc(ln2_g),
            "w1T": np.ascontiguousarray(np.asarray(w1, f).T).astype(NPBF16),
            "b1p": np.ascontiguousarray(
                np.asarray(b1 + ln2_b @ np.asarray(w1, f).T, f)
                .reshape(DFF // P, P).T).astype(f),
            "w2T": np.ascontiguousarray(np.asarray(w2, f).T).astype(NPBF16),
            "b2_bc": bc(b2),
        }
        in_maps.append(m)
    return in_maps


def kernel(**inputs):
    from concourse.bass_utils import run_bass_kernel_spmd
    if "nc" not in _CACHE:
        _CACHE["nc"] = build_nc()
    nc = _CACHE["nc"]
    in_maps = make_in_maps(**inputs)
    res = run_bass_kernel_spmd(nc, in_maps, list(range(8)))
    out = np.empty((B, S, D), np.float32)
    for core in range(8):
        b, g = core // G, core % G
        rows = res.results[core]["out_rows"]
        for sb in range(4):
            out[b, sb * 512 + g * P:sb * 512 + (g + 1) * P, :] = \
                rows[sb * P:(sb + 1) * P]
    return out


# revision 53
# speedup vs baseline: 1.8164x; 1.0514x over previous
"""Trainium2 Bass kernel: pre-LN transformer decoder layer on 8 NeuronCores.

Sharding: core = 4*b + g  (b in {0,1} batch, g in {0..3} group rank).
  - Attention: head-parallel (4 of 16 heads per core) over the full batch-b
    sequence, computed in a TRANSPOSED-score formulation:
      scores_T[k, q] = K_blk @ Q^T   (softmax needs no max-subtraction here;
      scores are O(6)), exp on ScalarE with the key-padding mask as a
      per-partition bias, denominator via an appended ones-column on V,
      so no P-transposes and no reductions at all.  attn output comes out
      already transposed [dh, q] — perfect for the output projection.
  - Output projection partials (4 local heads) are computed per 512-token
    group and exchanged with chunked bf16 ReduceScatters (1 MB each),
    overlapped with the attention compute of later token groups.
  - FFN: token-parallel (512 tokens per core, strided 128-blocks) with
    full weights (bf16).
All matmul operands are bf16 (fp32 accumulation in PSUM).
"""
import math
import numpy as np
import ml_dtypes

import concourse.bacc as bacc
import concourse.bass as bass
import concourse.tile as tile
from concourse import mybir
from concourse.masks import make_identity

B, S, D, H, DH, DFF = 2, 2048, 1024, 16, 64, 4096
G = 4            # cores per batch
LH = H // G      # local heads
LD = LH * DH     # 256 local head dims
SL = S // G      # 512 tokens per core for FFN / output projection
P = 128
NB = S // P      # 16 token blocks
DC = D // P      # 8 d chunks
F32 = mybir.dt.float32
F32R = mybir.dt.float32r
BF16 = mybir.dt.bfloat16
NEG = -1e9
NPBF16 = ml_dtypes.bfloat16

_CACHE = {}


def r(ap):
    return ap.bitcast(F32R)


def build_nc():
    nc = bacc.Bacc("TRN2", target_bir_lowering=False, debug=False, num_devices=8)
    d = {}
    def inp(name, shape, dt=F32):
        d[name] = nc.dram_tensor(name, list(shape), dt, kind="ExternalInput").ap()
    inp("xfull", (S, D))
    inp("xrows", (SL, D))
    inp("wqT", (D, LD), BF16); inp("wkT", (D, LD), BF16); inp("wvT", (D, LD), BF16)
    inp("qb", (P, 2)); inp("kb2", (P, 2)); inp("vb_bc", (P, LD))
    inp("wo_pair", (P, 2, D), BF16); inp("wob_bc", (P, D))
    inp("mdiagT", (P, NB, P)); inp("rmask16", (P, NB))
    inp("ln1g_bc", (P, D)); inp("ln2g_bc", (P, D))
    inp("w1T", (D, DFF), BF16); inp("b1p", (P, DFF // P))
    inp("w2T", (DFF, D), BF16); inp("b2_bc", (P, D))
    out_rows = nc.dram_tensor("out_rows", [SL, D], F32, kind="ExternalOutput").ap()
    partial_d = nc.dram_tensor("partial_d", [S, D], BF16).ap()
    rs_d = nc.dram_tensor("rs_d", [SL, D], BF16).ap()

    with tile.TileContext(nc) as tc:
        with tc.tile_pool(name="consts", bufs=1) as consts:
            identb = consts.tile([P, P], BF16)
            make_identity(nc, identb)
            eps_sb = consts.tile([P, 1], F32)
            nc.vector.memset(eps_sb, 1e-5)
            w1_cm = tc.tile_pool(name="w1p", bufs=1)
            w1_pool = w1_cm.__enter__()
            w1full = w1_pool.tile([P, DC, DFF], BF16)
            qt_cm = tc.tile_pool(name="qt", bufs=1)
            qt_pool = qt_cm.__enter__()
            QT = qt_pool.tile([P, 2, S], BF16)      # [dh-in-pair, pb, s] (q scaled+bias)
            KT = qt_pool.tile([P, 2, S], BF16)
            V = qt_pool.tile([P, NB, LH, DH + 1], BF16)   # token-major V + ones col
            nc.vector.memset(V[:, :, :, DH:DH + 1], 1.0)

            # ---------------- Phase A: LN1 + transpose, Phase B: QKV ----------
            with tc.tile_pool(name="ab", bufs=3) as ab, \
                 tc.tile_pool(name="abw", bufs=1) as abw, \
                 tc.tile_pool(name="xnt_p", bufs=1) as xnt_p, \
                 tc.tile_pool(name="abps", bufs=2, space="PSUM") as abps, \
                 tc.tile_pool(name="qkps", bufs=2, space="PSUM") as qkps:
                ln1g = abw.tile([P, D], F32); nc.sync.dma_start(out=ln1g, in_=d["ln1g_bc"][:])
                wq_sb = abw.tile([P, DC, LD], BF16)
                nc.sync.dma_start(out=wq_sb, in_=d["wqT"].rearrange("(c p) o -> p c o", p=P))
                wk_sb = abw.tile([P, DC, LD], BF16)
                nc.sync.dma_start(out=wk_sb, in_=d["wkT"].rearrange("(c p) o -> p c o", p=P))
                wv_sb = abw.tile([P, DC, LD], BF16)
                nc.sync.dma_start(out=wv_sb, in_=d["wvT"].rearrange("(c p) o -> p c o", p=P))
                qb_sb = abw.tile([P, 2], F32); nc.sync.dma_start(out=qb_sb, in_=d["qb"][:])
                kb_sb = abw.tile([P, 2], F32); nc.sync.dma_start(out=kb_sb, in_=d["kb2"][:])
                vb_sb = abw.tile([P, LD], F32); nc.sync.dma_start(out=vb_sb, in_=d["vb_bc"][:])
                XNT = xnt_p.tile([P, DC, S], BF16)

                for i in range(NB):
                    xin = ab.tile([P, D], F32, tag="xin")
                    nc.sync.dma_start(out=xin, in_=d["xfull"][i * P:(i + 1) * P, :])
                    stats = ab.tile([P, 2, 6], F32, tag="st")
                    nc.vector.bn_stats(out=stats[:, 0, :], in_=xin[:, 0:512])
                    nc.vector.bn_stats(out=stats[:, 1, :], in_=xin[:, 512:1024])
                    mv = ab.tile([P, 2], F32, tag="mv")
                    nc.vector.bn_aggr(out=mv, in_=stats)
                    rs_t = ab.tile([P, 1], F32, tag="rs")
                    nc.scalar.activation(out=rs_t, in_=mv[:, 1:2],
                                         func=mybir.ActivationFunctionType.Sqrt,
                                         bias=eps_sb)
                    nc.vector.reciprocal(out=rs_t, in_=rs_t)
                    nmurs = ab.tile([P, 1], F32, tag="nm")
                    nc.vector.tensor_tensor(out=nmurs, in0=mv[:, 0:1], in1=rs_t,
                                            op=mybir.AluOpType.mult)
                    nc.vector.tensor_scalar_mul(nmurs, nmurs, -1.0)
                    xn = ab.tile([P, D], F32, tag="xn")
                    nc.scalar.activation(out=xn, in_=xin,
                                         func=mybir.ActivationFunctionType.Identity,
                                         scale=rs_t, bias=nmurs)
                    xnb = ab.tile([P, D], BF16, tag="xnb")
                    nc.vector.tensor_tensor(out=xnb, in0=xn, in1=ln1g,
                                            op=mybir.AluOpType.mult)
                    for j in range(DC // 2):
                        pt = abps.tile([P, 2, P], BF16, tag="tp")
                        nc.tensor.transpose(pt[:, 0, :], xnb[:, 2 * j * P:(2 * j + 1) * P], identb)
                        nc.tensor.transpose(pt[:, 1, :], xnb[:, (2 * j + 1) * P:(2 * j + 2) * P], identb)
                        nc.any.tensor_copy(out=XNT[:, 2 * j:2 * j + 2, i * P:(i + 1) * P],
                                           in_=pt)

                # prefetch FFN w1 weights now that the input-stream DMAs are issued
                w1rr = d["w1T"].rearrange("(c p) m -> p c m", p=P)
                for c in range(DC):
                    nc.sync.dma_start(out=w1full[:, c, :], in_=w1rr[:, c, :])

                # QKV projections
                for pb in range(2):
                    for sc in range(S // 512):
                        psq = qkps.tile([P, 512], F32, tag="psq")
                        psk = qkps.tile([P, 512], F32, tag="psk")
                        for dc in range(DC):
                            nc.tensor.matmul(psq, wq_sb[:, dc, pb * P:(pb + 1) * P],
                                             XNT[:, dc, sc * 512:(sc + 1) * 512],
                                             start=(dc == 0), stop=(dc == DC - 1))
                        for dc in range(DC):
                            nc.tensor.matmul(psk, wk_sb[:, dc, pb * P:(pb + 1) * P],
                                             XNT[:, dc, sc * 512:(sc + 1) * 512],
                                             start=(dc == 0), stop=(dc == DC - 1))
                        nc.scalar.activation(out=QT[:, pb, sc * 512:(sc + 1) * 512],
                                             in_=psq,
                                             func=mybir.ActivationFunctionType.Identity,
                                             bias=qb_sb[:, pb:pb + 1])
                        nc.scalar.activation(out=KT[:, pb, sc * 512:(sc + 1) * 512],
                                             in_=psk,
                                             func=mybir.ActivationFunctionType.Identity,
                                             bias=kb_sb[:, pb:pb + 1])
                for sb in range(NB):
                    psv = qkps.tile([P, LD], F32, tag="psv")
                    for dc in range(DC):
                        nc.tensor.matmul(psv, XNT[:, dc, sb * P:(sb + 1) * P],
                                         wv_sb[:, dc, :],
                                         start=(dc == 0), stop=(dc == DC - 1))
                    nc.vector.tensor_tensor(
                        out=V[:, sb, :, 0:DH],
                        in0=psv[:].rearrange("p (h e) -> p h e", h=LH),
                        in1=vb_sb[:].rearrange("p (h e) -> p h e", h=LH),
                        op=mybir.AluOpType.add)

            # ---------------- Phase C: attention (transposed scores) ----------
            with tc.tile_pool(name="cw", bufs=1) as cw, \
                 tc.tile_pool(name="cat", bufs=2) as cat, \
                 tc.tile_pool(name="cee", bufs=3) as cee, \
                 tc.tile_pool(name="csm", bufs=2) as csm, \
                 tc.tile_pool(name="c_ps_s", bufs=2, space="PSUM") as c_ps_s, \
                 tc.tile_pool(name="c_ps_at", bufs=1, space="PSUM") as c_ps_at:
                wo_sb = cw.tile([P, 2, D], BF16)
                nc.sync.dma_start(out=wo_sb, in_=d["wo_pair"][:])
                mdT = cw.tile([P, NB, P], F32)
                nc.sync.dma_start(out=mdT, in_=d["mdiagT"][:])
                rm_sb = cw.tile([P, NB], F32)
                nc.sync.dma_start(out=rm_sb, in_=d["rmask16"][:])

                for gq in range(4):
                    ATg = cat.tile([P, 2, 512], BF16, tag="ATg")
                    for pb in range(2):   # head pair (2*pb, 2*pb+1), row-packed
                        at0 = c_ps_at.tile([DH + 1, 512], F32, tag=f"at{pb}0",
                                           name="at0")
                        at1 = c_ps_at.tile([DH + 1, 512], F32, tag=f"at{pb}1",
                                           name="at1")
                        ats = (at0, at1)
                        for kb in range(4 * gq + 4):
                            q0, qe = max(kb, 4 * gq), 4 * gq + 4
                            w = (qe - q0) * P
                            col0 = (q0 - 4 * gq) * P
                            ps0 = c_ps_s.tile([P, 512], F32, tag="ps0")
                            ps1 = c_ps_s.tile([P, 512], F32, tag="ps1")
                            pss = (ps0, ps1)
                            # two heads in different PE row-groups -> concurrent
                            for i, po in enumerate((0, 64)):
                                nc.tensor.matmul(
                                    pss[i][:, :w],
                                    KT[po:po + 64, pb, kb * P:(kb + 1) * P],
                                    QT[po:po + 64, pb, q0 * P:qe * P],
                                    start=True, stop=True)
                            for i in range(2):
                                if q0 == kb:
                                    nc.vector.tensor_tensor(out=pss[i][:, 0:P],
                                                            in0=pss[i][:, 0:P],
                                                            in1=mdT[:, kb, :],
                                                            op=mybir.AluOpType.add)
                                ept = cee.tile([P, 512], BF16, tag="ept")
                                nc.scalar.activation(
                                    out=ept[:, :w], in_=pss[i][:, :w],
                                    func=mybir.ActivationFunctionType.Exp,
                                    bias=rm_sb[:, kb:kb + 1])
                                Vst = V[:, kb, 2 * pb + i, :]
                                if q0 == kb:
                                    nc.tensor.matmul(ats[i][:, col0:col0 + P], Vst,
                                                     ept[:, 0:P],
                                                     start=(kb == 0), stop=True)
                                    if w > P:
                                        nc.tensor.matmul(ats[i][:, col0 + P:col0 + w],
                                                         Vst, ept[:, P:w],
                                                         start=(kb == 0), stop=False)
                                else:
                                    nc.tensor.matmul(ats[i][:, 0:w], Vst, ept[:, :w],
                                                     start=(kb == 0), stop=False)
                        for i in range(2):
                            rinv = csm.tile([1, 512], F32, tag="ri")
                            nc.vector.reciprocal(out=rinv, in_=ats[i][DH:DH + 1, :])
                            rb_sb = csm.tile([64, 512], F32, tag="rbs")
                            nc.gpsimd.partition_broadcast(rb_sb, rinv[0:1, :],
                                                          channels=64)
                            nc.vector.tensor_tensor(out=ATg[i * 64:(i + 1) * 64, pb, :],
                                                    in0=ats[i][0:DH, :], in1=rb_sb,
                                                    op=mybir.AluOpType.mult)
                    # output-projection partials for this token group (bf16)
                    for qi2 in range(4):
                        row0 = (gq * 4 + qi2) * P
                        for oc in range(2):
                            pp = c_ps_s.tile([P, 512], F32, tag="ps0", name="pp")
                            for hp in range(2):
                                nc.tensor.matmul(
                                    pp, ATg[:, hp, qi2 * P:(qi2 + 1) * P],
                                    wo_sb[:, hp, oc * 512:(oc + 1) * 512],
                                    start=(hp == 0), stop=(hp == 1))
                            ppb = cee.tile([P, 512], BF16, tag="ppb")
                            nc.any.tensor_copy(out=ppb, in_=pp)
                            nc.sync.dma_start(
                                out=partial_d[row0:row0 + P, oc * 512:(oc + 1) * 512],
                                in_=ppb)
                    nc.gpsimd.collective_compute(
                        "ReduceScatter", mybir.AluOpType.add,
                        replica_groups=[[0, 1, 2, 3], [4, 5, 6, 7]],
                        ins=[partial_d[gq * 512:(gq + 1) * 512, :]],
                        outs=[rs_d[gq * P:(gq + 1) * P, :]])

            qt_cm.__exit__(None, None, None)

            # ---------------- Phase D: output proj + residual + LN2 + FFN -----
            with tc.tile_pool(name="dw", bufs=1) as dw, \
                 tc.tile_pool(name="dt", bufs=3) as dt, \
                 tc.tile_pool(name="dxp", bufs=1) as dxp, \
                 tc.tile_pool(name="dh", bufs=1) as dh_p, \
                 tc.tile_pool(name="dw2", bufs=6) as dw2_p, \
                 tc.tile_pool(name="d_ps_h", bufs=2, space="PSUM") as d_ps_h, \
                 tc.tile_pool(name="d_ps_t", bufs=2, space="PSUM") as d_ps_t, \
                 tc.tile_pool(name="d_ps_o", bufs=1, space="PSUM") as d_ps_o:
                xr_sb = dw.tile([P, 4, D], F32)
                nc.sync.dma_start(out=xr_sb, in_=d["xrows"].rearrange("(s p) d -> p s d", p=P))
                wob = dw.tile([P, D], F32); nc.sync.dma_start(out=wob, in_=d["wob_bc"][:])
                ln2g = dw.tile([P, D], F32); nc.sync.dma_start(out=ln2g, in_=d["ln2g_bc"][:])
                b1_sb = dw.tile([P, DFF // P], F32); nc.sync.dma_start(out=b1_sb, in_=d["b1p"][:])
                b2_sb = dw.tile([P, D], F32); nc.sync.dma_start(out=b2_sb, in_=d["b2_bc"][:])
                XP = dxp.tile([P, 4, D], F32)   # X' rows (post-attn residual)
                YNT = dxp.tile([P, DC, SL], BF16)
                HT = dh_p.tile([P, DFF // P, SL], BF16)

                for sb in range(4):
                    rs_sb = dt.tile([P, D], BF16, tag="rs_in")
                    nc.sync.dma_start(out=rs_sb, in_=rs_d[sb * P:(sb + 1) * P, :])
                    tmp = dt.tile([P, D], F32, tag="wtmp")
                    nc.vector.tensor_tensor(out=tmp, in0=rs_sb, in1=xr_sb[:, sb, :],
                                            op=mybir.AluOpType.add)
                    nc.vector.tensor_tensor(out=XP[:, sb, :], in0=tmp, in1=wob,
                                            op=mybir.AluOpType.add)
                    stats = dt.tile([P, 2, 6], F32, tag="st2")
                    nc.vector.bn_stats(out=stats[:, 0, :], in_=XP[:, sb, 0:512])
                    nc.vector.bn_stats(out=stats[:, 1, :], in_=XP[:, sb, 512:1024])
                    mv = dt.tile([P, 2], F32, tag="mv2")
                    nc.vector.bn_aggr(out=mv, in_=stats)
                    rs_t = dt.tile([P, 1], F32, tag="rs2")
                    nc.scalar.activation(out=rs_t, in_=mv[:, 1:2],
                                         func=mybir.ActivationFunctionType.Sqrt,
                                         bias=eps_sb)
                    nc.vector.reciprocal(out=rs_t, in_=rs_t)
                    nmurs = dt.tile([P, 1], F32, tag="nm2")
                    nc.vector.tensor_tensor(out=nmurs, in0=mv[:, 0:1], in1=rs_t,
                                            op=mybir.AluOpType.mult)
                    nc.vector.tensor_scalar_mul(nmurs, nmurs, -1.0)
                    yn = dt.tile([P, D], F32, tag="yn")
                    nc.scalar.activation(out=yn, in_=XP[:, sb, :],
                                         func=mybir.ActivationFunctionType.Identity,
                                         scale=rs_t, bias=nmurs)
                    ynb = dt.tile([P, D], BF16, tag="ynb")
                    nc.vector.tensor_tensor(out=ynb, in0=yn, in1=ln2g,
                                            op=mybir.AluOpType.mult)
                    for j in range(DC // 2):
                        tp = d_ps_t.tile([P, 2, P], BF16, tag="tp3")
                        nc.tensor.transpose(tp[:, 0, :], ynb[:, 2 * j * P:(2 * j + 1) * P], identb)
                        nc.tensor.transpose(tp[:, 1, :], ynb[:, (2 * j + 1) * P:(2 * j + 2) * P], identb)
                        nc.any.tensor_copy(out=YNT[:, 2 * j:2 * j + 2, sb * P:(sb + 1) * P],
                                           in_=tp)

                w2r = d["w2T"].rearrange("(c p) o -> p c o", p=P)
                for c in range(DFF // P):
                    ps_h = d_ps_h.tile([P, SL], F32, tag="ps_h")
                    for dc in range(DC):
                        nc.tensor.matmul(ps_h, w1full[:, dc, c * P:(c + 1) * P],
                                         YNT[:, dc, :],
                                         start=(dc == 0), stop=(dc == DC - 1))
                    nc.scalar.activation(out=HT[:, c, :], in_=ps_h,
                                         func=mybir.ActivationFunctionType.Gelu,
                                         bias=b1_sb[:, c:c + 1])

                for oc in range(2):
                    ps_os = [d_ps_o.tile([P, 512], F32, tag=f"ps_o{sb}", name=f"ps_o{sb}")
                             for sb in range(4)]
                    for c in range(DFF // P):
                        w2c = dw2_p.tile([P, 512], BF16, tag="w2c")
                        nc.sync.dma_start(out=w2c,
                                          in_=w2r[:, c, oc * 512:(oc + 1) * 512])
                        for sb in range(4):
                            nc.tensor.matmul(
                                ps_os[sb], HT[:, c, sb * P:(sb + 1) * P],
                                w2c,
                                start=(c == 0), stop=(c == DFF // P - 1))
                    for sb in range(4):
                        fin = dt.tile([P, 512], F32, tag="fin")
                        nc.vector.tensor_tensor(out=fin, in0=ps_os[sb],
                                                in1=b2_sb[:, oc * 512:(oc + 1) * 512],
                                                op=mybir.AluOpType.add)
                        nc.vector.tensor_tensor(out=fin, in0=fin,
                                                in1=XP[:, sb, oc * 512:(oc + 1) * 512],
                                                op=mybir.AluOpType.add)
                        nc.sync.dma_start(
                            out=out_rows[sb * P:(sb + 1) * P, oc * 512:(oc + 1) * 512],
                            in_=fin)

            w1_cm.__exit__(None, None, None)

    nc.compile()
    return nc


def make_in_maps(X, mask, valid_lens, wq_w, wq_b, wk_w, wv_w, wv_b, wo_w, wo_b,
                 ln1_g, ln1_b, ln2_g, ln2_b, w1, b1, w2, b2):
    f = np.float32
    bc = lambda v: np.broadcast_to(np.asarray(v, f)[None, :], (P, len(v))).copy()
    # transposed causal diagonal blocks: mdT[kk, i, qq] = mask[i*P+qq, i*P+kk]
    mdT = np.stack([np.asarray(mask[i * P:(i + 1) * P, i * P:(i + 1) * P]).T
                    for i in range(NB)])            # [i, kk, qq]
    mdT = np.ascontiguousarray(mdT.transpose(1, 0, 2)).astype(f)   # [kk, i, qq]
    idx = np.arange(S)
    woT = np.ascontiguousarray(np.asarray(wo_w, f).T)   # [d_in, d_out]
    p_ar = np.arange(P)
    in_maps = []
    for core in range(8):
        b, g = core // G, core % G
        vmask1 = np.where(idx >= valid_lens[b], NEG, 0.0).astype(f)
        hs = slice(g * LD, (g + 1) * LD)
        # local-head-pair layout matching ATg: partition p = (h%2)*64 + dh,
        # second dim hp = h//2, local head h = hp*2 + p//64
        wo_pair = np.empty((P, 2, D), NPBF16)
        for hp in range(2):
            head = g * 4 + hp * 2 + p_ar // 64
            rows = head * 64 + (p_ar % 64)
            wo_pair[:, hp, :] = woT[rows, :].astype(NPBF16)
        # strided token ownership: rows sb*512 + g*128 .. +128, sb = 0..3
        own = np.concatenate([np.arange(sb * 512 + g * P, sb * 512 + (g + 1) * P)
                              for sb in range(4)])
        m = {
            "xfull": np.ascontiguousarray(X[b]).astype(f),
            "xrows": np.ascontiguousarray(X[b][own]).astype(f),
            "wqT": np.ascontiguousarray((wq_w[hs, :] * 0.125).T).astype(NPBF16),
            "wkT": np.ascontiguousarray(wk_w[hs, :].T).astype(NPBF16),
            "wvT": np.ascontiguousarray(wv_w[hs, :].T).astype(NPBF16),
            "qb": np.ascontiguousarray(
                (0.125 * (wq_b[hs] + ln1_b @ np.asarray(wq_w, f)[hs].T))
                .reshape(2, P).T).astype(f),
            "kb2": np.ascontiguousarray(
                (ln1_b @ np.asarray(wk_w, f)[hs].T).reshape(2, P).T).astype(f),
            "vb_bc": bc(wv_b[hs] + ln1_b @ np.asarray(wv_w, f)[hs].T),
            "wo_pair": wo_pair,
            "wob_bc": bc(wo_b),
            "mdiagT": mdT,
            "rmask16": np.ascontiguousarray(vmask1.reshape(NB, P).T).astype(f),
            "ln1g_bc": bc(ln1_g), "ln2g_bc": bc(ln2_g),
            "w1T": np.ascontiguousarray(np.asarray(w1, f).T).astype(NPBF16),
            "b1p": np.ascontiguousarray(
                np.asarray(b1 + ln2_b @ np.asarray(w1, f).T, f)
                .reshape(DFF // P, P).T).astype(f),
            "w2T": np.ascontiguousarray(np.asarray(w2, f).T).astype(NPBF16),
            "b2_bc": bc(b2),
        }
        in_maps.append(m)
    return in_maps


def kernel(**inputs):
    from concourse.bass_utils import run_bass_kernel_spmd
    if "nc" not in _CACHE:
        _CACHE["nc"] = build_nc()
    nc = _CACHE["nc"]
    in_maps = make_in_maps(**inputs)
    res = run_bass_kernel_spmd(nc, in_maps, list(range(8)))
    out = np.empty((B, S, D), np.float32)
    for core in range(8):
        b, g = core // G, core % G
        rows = res.results[core]["out_rows"]
        for sb in range(4):
            out[b, sb * 512 + g * P:sb * 512 + (g + 1) * P, :] = \
                rows[sb * P:(sb + 1) * P]
    return out
